# revision 25
# baseline (speedup 1.0000x reference)
"""Multi-head attention (RoPE-full-dmodel variant) on 8 TRN2 NeuronCores.

Sharding: core c = (batch c//4, head-group c%4 of 4 heads).
 - W_q/W_k/W_v split column-wise by head (each core projects its 256 channels)
 - W_o split row-wise; per-core partial outputs summed on host (all-reduce at gather)

Perf-tuned fp16 design (394us baseline -> ~268us measured, NTFF-profiled):
 - Scores use 128-partition contraction: per-head K tiles zero-padded to 128
   partitions (K=64 matmuls run at ~half the row rate of K=128 on TRN2; the
   packed-Q rhs upper/lower half is nullified by the zeros).  kt tiles reuse
   the vch[4..7] SBUF slots (dead after the V projection) via pool-tag rotation.
 - DMA priority order (k+tables first for the DVE RoPE critical path, V path
   interleaved), half-S granularity; weights host-prepacked to SBUF layout.
 - RoPE quarter/half-S granular, in-place (roped halves overwrite inputs),
   pipelined with K/Q projections; kproj/qproj for s-blocks 2-3 are hooked
   into the attention MM stream.
 - Attention inner loop 3-ahead score pipelining ([128,512] single-bank score
   tiles; PSUM: 3 sc + 3 ut + 2 gen = 8 banks) so PV never waits on exp.
 - Softmax denominators via ones-column; deferred normalization (next head's
   prologue) keeps the norm chain off the PE critical path; ut split per
   query-block avoids coarse-range false deps on out-proj.
 - V bias is folded on host (attn rows sum to 1 => U = attn@V + bv exactly).
 - Out-proj hooked into att(qbp1); the last head runs half-major so the final
   norms overlap compute; staging copies ride the ACT engine.
"""
import os
import sys
from contextlib import nullcontext

for _p in ("/opt/trn_rl_repo", "/root/.axon_site/_ro/trn_rl_repo"):
    if os.path.isdir(_p) and _p not in sys.path:
        sys.path.insert(0, _p)

import numpy as np

import concourse.bacc as bacc
import concourse.tile as tile
import concourse.mybir as mybir
from concourse.bass_utils import run_bass_kernel_spmd

B, S, D = 2, 2048, 1024
H_TOT, DK = 16, 64
N_CORES, GROUPS = 8, 4
CH = 256            # channels (heads*dk) per core
KC = D // 128       # 8 d-model chunks
QB = S // 512       # 4 query s-blocks
ST = S // 128       # 16 seq tiles
HPC = 4             # heads per core
BASE = 10000.0

MM = mybir.dt.float16
F32 = mybir.dt.float32
AF = mybir.ActivationFunctionType

# even/odd permutation of the d_model axis: row r <- old d = 2r (r<512), 2(r-512)+1
_PERM = np.concatenate([np.arange(0, D, 2), np.arange(1, D, 2)])

_PROG = None


def _build(loop_n=1):
    nc = bacc.Bacc("TRN2", target_bir_lowering=False, debug=False)
    qT = nc.dram_tensor("qT", (D, S), MM, kind="ExternalInput").ap()
    kT = nc.dram_tensor("kT", (D, S), MM, kind="ExternalInput").ap()
    vT = nc.dram_tensor("vT", (D, S), MM, kind="ExternalInput").ap()
    cosc = nc.dram_tensor("cosc", (D // 2, S), MM, kind="ExternalInput").ap()
    sinc = nc.dram_tensor("sinc", (D // 2, S), MM, kind="ExternalInput").ap()
    wq = nc.dram_tensor("wq", (128, KC * CH), MM, kind="ExternalInput").ap()
    wk = nc.dram_tensor("wk", (128, KC * CH), MM, kind="ExternalInput").ap()
    wv = nc.dram_tensor("wv", (128, KC * CH), MM, kind="ExternalInput").ap()
    wo = nc.dram_tensor("wo", (128, 2 * D), MM, kind="ExternalInput").ap()
    bq = nc.dram_tensor("bq", (2, 128, 1), F32, kind="ExternalInput").ap()
    bk = nc.dram_tensor("bk", (2, 128, 1), F32, kind="ExternalInput").ap()
    ident = nc.dram_tensor("ident", (128, 128), MM, kind="ExternalInput").ap()
    out = nc.dram_tensor("out", (S, D), MM, kind="ExternalOutput").ap()

    with tile.TileContext(nc) as tc:
      with (
          tc.tile_pool(name="consts", bufs=1) as consts,
          tc.tile_pool(name="qkv", bufs=1) as qkv,
          tc.tile_pool(name="xin", bufs=1) as xin,
          tc.tile_pool(name="vc", bufs=1) as vc,
          tc.tile_pool(name="tmp", bufs=4) as tmpp,
          tc.tile_pool(name="expp", bufs=4) as expp,
          tc.tile_pool(name="misc", bufs=2) as misc,
          tc.tile_pool(name="outst", bufs=2) as outst,
          tc.tile_pool(name="ps_gen", bufs=2, space="PSUM") as ps_gen,
          tc.tile_pool(name="ps_sc", bufs=3, space="PSUM") as ps_sc,
          tc.tile_pool(name="ps_ut", bufs=3, space="PSUM") as ps_ut,
      ):
        with (tc.For_i(0, loop_n, 1, hint_engines=tuple(mybir.ALL_ENGINES))
              if loop_n > 1 else nullcontext()):
            # ---------------- static tiles ----------------
            wv_sb = consts.tile([128, KC * CH], MM, tag="wv")
            wk_sb = consts.tile([128, KC * CH], MM, tag="wk")
            wq_sb = consts.tile([128, KC * CH], MM, tag="wq")
            wo_sb = consts.tile([128, 2 * D], MM, tag="wo")
            ident_sb = consts.tile([128, 128], MM, tag="ident")
            bq_sb, bk_sb = [], []
            for c in range(2):
                t_ = consts.tile([128, 1], F32, tag=f"bq{c}", name=f"bq{c}")
                bq_sb.append(t_)
                t_ = consts.tile([128, 1], F32, tag=f"bk{c}", name=f"bk{c}")
                bk_sb.append(t_)

            v_sb = qkv.tile([128, ST * HPC * 65], MM, tag="v")
            ones_cols = v_sb[:].rearrange("p (b c) -> p b c", c=65)[:, :, 64]
            nc.vector.memset(ones_cols, 1.0)

            qt_sb = [qkv.tile([128, S], MM, tag=f"qt{c}", name=f"qt{c}") for c in range(2)]
            utq = [[qkv.tile([128, 1024], MM, tag=f"ut{q}{c}", name=f"ut{q}{c}")
                    for c in range(2)] for q in range(2)]
            cos_sb = [qkv.tile([128, S], MM, tag=f"cos{t}", name=f"cos{t}") for t in range(4)]
            sin_sb = [qkv.tile([128, S], MM, tag=f"sin{t}", name=f"sin{t}") for t in range(4)]
            xk = [xin.tile([128, S], MM, tag=f"xk{a}", name=f"xk{a}") for a in range(KC)]
            xq = [xin.tile([128, S], MM, tag=f"xq{a}", name=f"xq{a}") for a in range(KC)]
            vch = [vc.tile([128, S], MM, tag=f"vch{d}", name=f"vch{d}") for d in range(KC)]
            vt_cm = [vc.tile([128, S], MM, tag=f"vtc{c}", name=f"vtc{c}") for c in range(2)]

            # ---------------- DMA emission (priority order) ----------------
            HLV = ((0, 1024), (1024, 2048))

            def dma_half(dst_tile, src_rows, lo, hi):
                nc.sync.dma_start(dst_tile[:, lo:hi], src_rows[:, lo:hi])

            # weights prepacked on host into the SBUF layout (contiguous DMA)
            nc.sync.dma_start(wv_sb[:], wv)
            nc.sync.dma_start(ident_sb[:], ident)
            lo, hi = HLV[0]
            # K-path pair (0,2) + tables, then V half0, then rest of K half0
            for a in (0, 2):
                dma_half(xk[a], kT[128 * a:128 * (a + 1), :], lo, hi)
            for t in (0, 2):
                dma_half(cos_sb[t], cosc[128 * t:128 * (t + 1), :], lo, hi)
                dma_half(sin_sb[t], sinc[128 * t:128 * (t + 1), :], lo, hi)
            for d in range(KC):
                dma_half(vch[d], vT[128 * d:128 * (d + 1), :], lo, hi)
            for a in (1, 3):
                dma_half(xk[a], kT[128 * a:128 * (a + 1), :], lo, hi)
            for t in (1, 3):
                dma_half(cos_sb[t], cosc[128 * t:128 * (t + 1), :], lo, hi)
                dma_half(sin_sb[t], sinc[128 * t:128 * (t + 1), :], lo, hi)
            for a in (4, 6, 5, 7):
                dma_half(xk[a], kT[128 * a:128 * (a + 1), :], lo, hi)
            lo1, hi1 = HLV[1]
            for d in range(KC):
                dma_half(vch[d], vT[128 * d:128 * (d + 1), :], lo1, hi1)
            for a in (0, 2, 1, 3, 4, 6, 5, 7):
                dma_half(xq[a], qT[128 * a:128 * (a + 1), :], lo, hi)
            nc.sync.dma_start(wk_sb[:], wk)
            nc.sync.dma_start(wq_sb[:], wq)
            for c in range(2):
                nc.sync.dma_start(bq_sb[c][:], bq[c])
                nc.sync.dma_start(bk_sb[c][:], bk[c])
            for a in (0, 2, 1, 3, 4, 6, 5, 7):
                dma_half(xk[a], kT[128 * a:128 * (a + 1), :], lo1, hi1)
            for t in (0, 2, 1, 3):
                dma_half(cos_sb[t], cosc[128 * t:128 * (t + 1), :], lo1, hi1)
                dma_half(sin_sb[t], sinc[128 * t:128 * (t + 1), :], lo1, hi1)
            for a in (0, 2, 1, 3, 4, 6, 5, 7):
                dma_half(xq[a], qT[128 * a:128 * (a + 1), :], lo1, hi1)
            nc.sync.dma_start(wo_sb[:], wo)

            # ---------------- helpers ----------------
            def rope_cols(x, lo, hi):
                """RoPE in place on x[0..7] over columns [lo,hi) (DVE)."""
                w = hi - lo
                for a in (0, 1, 4, 5):
                    b_ = a + 2
                    ca, cb = a % 4, b_ % 4
                    xa, xb = x[a], x[b_]
                    t1 = tmpp.tile([128, w], MM, tag="t", name="t1")
                    nc.vector.tensor_mul(t1[:], xa[:, lo:hi], cos_sb[ca][:, lo:hi])
                    t2 = tmpp.tile([128, w], MM, tag="t", name="t2")
                    nc.vector.tensor_mul(t2[:], xb[:, lo:hi], sin_sb[ca][:, lo:hi])
                    t3 = tmpp.tile([128, w], MM, tag="t", name="t3")
                    nc.vector.tensor_mul(t3[:], xb[:, lo:hi], cos_sb[cb][:, lo:hi])
                    t4 = tmpp.tile([128, w], MM, tag="t", name="t4")
                    nc.vector.tensor_mul(t4[:], xa[:, lo:hi], sin_sb[cb][:, lo:hi])
                    nc.vector.tensor_sub(xa[:, lo:hi], t1[:], t2[:])
                    nc.vector.tensor_add(xb[:, lo:hi], t3[:], t4[:])

            def proj_block(w_sb, b_sb, src, dst, s, khead=False):
                """One 512-col projection s-block: psum over 8 d-chunks, ACT out.

                khead: dst is the list of 4 zero-padded per-head kt tiles; write
                the two 64-channel head halves separately (same partitions)."""
                for c in range(2):
                    ps = ps_gen.tile([128, 512], F32, tag="proj", name="psum")
                    for d in range(KC):
                        lhsT = w_sb[:, CH * d + 128 * c: CH * d + 128 * (c + 1)]
                        nc.tensor.matmul(ps[:], lhsT, src[d][:, 512 * s:512 * (s + 1)],
                                         start=(d == 0), stop=(d == KC - 1))
                    if khead:
                        sl = slice(512 * s, 512 * (s + 1))
                        nc.scalar.activation(dst[2 * c][0:64, sl], ps[0:64, :],
                                             AF.Identity, bias=b_sb[c][0:64])
                        nc.scalar.activation(dst[2 * c + 1][64:128, sl], ps[64:128, :],
                                             AF.Identity, bias=b_sb[c][64:128])
                    elif b_sb is None:
                        nc.scalar.copy(dst[c][:, 512 * s:512 * (s + 1)], ps[:])
                    else:
                        nc.scalar.activation(dst[c][:, 512 * s:512 * (s + 1)], ps[:],
                                             AF.Identity, bias=b_sb[c][:])

            def transpose_tile(t):
                pv = ps_gen.tile([128, CH], MM, tag="proj", name="pv")
                for c in range(2):
                    nc.tensor.transpose(pv[:, 128 * c:128 * (c + 1)],
                                        vt_cm[c][:, 128 * t:128 * (t + 1)],
                                        ident_sb[:])
                dst = _v_scatter_ap(v_sb, t)
                nc.scalar.copy(dst, pv[:])

            def outproj_block(st, tail=False):
                pos = [ps_gen.tile([128, 512], F32, tag="proj", name="po_")
                       for _ in range(2)]
                qbp_, loc = st // 8, 128 * (st % 8)
                for cc in range(2):
                    lhsT = utq[qbp_][cc][:, loc:loc + 128]
                    for nb in range(2):
                        nc.tensor.matmul(
                            pos[nb][:], lhsT,
                            wo_sb[:, D * cc + 512 * nb: D * cc + 512 * (nb + 1)],
                            start=(cc == 0), stop=(cc == 1))
                for nb in range(2):
                    stg = outst.tile([128, 512], MM, tag="stg", name="stg")
                    nc.scalar.copy(stg[:], pos[nb][:])
                    if tail:
                        for hh in range(2):
                            nc.sync.dma_start(
                                out[128 * st:128 * (st + 1),
                                    512 * nb + 256 * hh:512 * nb + 256 * (hh + 1)],
                                stg[:, 256 * hh:256 * (hh + 1)])
                    else:
                        nc.sync.dma_start(
                            out[128 * st:128 * (st + 1), 512 * nb:512 * (nb + 1)],
                            stg[:])

            # ---------------- V path (PE-first work) ----------------
            proj_block(wv_sb, None, vch, vt_cm, 0)
            proj_block(wv_sb, None, vch, vt_cm, 1)
            for t in range(8):
                transpose_tile(t)
            proj_block(wv_sb, None, vch, vt_cm, 2)
            proj_block(wv_sb, None, vch, vt_cm, 3)
            for t in range(8, 16):
                transpose_tile(t)

            # ---------------- RoPE + K/Q proj, S-half pipelined ----------------
            # Per-head K tiles, contraction zero-padded to 128 partitions:
            # K=64 matmuls run at ~half the row rate of K=128, so scores use
            # 128-partition lhsT with the unused head-half zeroed.  The rhs
            # (packed Q) upper/lower half carries the other head's data,
            # nullified by the zeros.  Tiles reuse the vch[4..7] SBUF (dead
            # after the V projection) via pool-tag rotation.
            kt4 = [vc.tile([128, S], MM, tag=f"vch{4 + h}", name=f"kt4_{h}")
                   for h in range(HPC)]
            for h in range(HPC):
                pad = slice(64, 128) if h % 2 == 0 else slice(0, 64)
                nc.gpsimd.memset(kt4[h][pad, :], 0.0)
            rope_cols(xk, 0, 512)
            rope_cols(xk, 512, 1024)
            proj_block(wk_sb, bk_sb, xk, kt4, 0, khead=True)
            proj_block(wk_sb, bk_sb, xk, kt4, 1, khead=True)
            rope_cols(xq, 0, 512)
            rope_cols(xq, 512, 1024)
            proj_block(wq_sb, bq_sb, xq, qt_sb, 0)
            proj_block(wq_sb, bq_sb, xq, qt_sb, 1)
            rope_cols(xk, 1024, 1536)
            rope_cols(xk, 1536, 2048)
            rope_cols(xq, 1024, 2048)

            # ---------------- attention ----------------
            def norm_head(pend, halves=(0, 1)):
                """Normalize a finished head's PV accumulators into utq."""
                qbp, ct, po, puts, uraws = pend
                for half in halves:
                    if uraws[half] is None:
                        u_ = misc.tile([65, 512], F32, tag="uraw", name="uraw")
                        nc.vector.tensor_copy(u_[:], puts[half][:])
                        uraws[half] = u_
                for half in halves:
                    uraw = uraws[half]
                    rec = misc.tile([1, 512], F32, tag="rec", name="rec")
                    nc.vector.reciprocal(rec[:], uraw[64:65, :])
                    bc = misc.tile([64, 512], F32, tag="bc", name="bc")
                    nc.gpsimd.partition_broadcast(bc[:], rec[:])
                    dst = utq[qbp][ct][po:po + 64, 512 * half:512 * (half + 1)]
                    nc.vector.tensor_mul(dst, uraw[0:64, :], bc[:])

            def attention(qbp, hooks=None, split_last=False):
                """hooks: dict (h, u) -> callable. Returns the last head's
                pending norm (emitted by the caller)."""
                hooks = hooks or {}
                q_lo = 1024 * qbp
                pending = None
                for h in range(HPC):
                    ct, po = h // 2, 64 * (h % 2)
                    qt_h = qt_sb[ct][:, q_lo:q_lo + 1024]
                    kt_h = kt4[h]
                    split = split_last and h == HPC - 1
                    order = ([2 * t for t in range(16)] + [2 * t + 1 for t in range(16)]
                             if split else list(range(32)))
                    puts = [ps_ut.tile([65, 512], F32, tag="ut", name=f"put{half}")
                            for half in range(2)]
                    pend_h = (qbp, ct, po, puts, [None, None])
                    es = [None] * 32

                    def sc_exp(u):
                        t, half = u // 2, u % 2
                        psc = ps_sc.tile([128, 512], F32, tag="sc", name="psc")
                        nc.tensor.matmul(psc[:],
                                         kt_h[:, 128 * t:128 * (t + 1)],
                                         qt_h[:, 512 * half:512 * (half + 1)],
                                         start=True, stop=True)
                        e = expp.tile([128, 512], MM, tag="e", name="e")
                        nc.scalar.activation(e[:], psc[:], AF.Exp, scale=0.125)
                        es[u] = e

                    sc_exp(order[0]); sc_exp(order[1]); sc_exp(order[2])
                    if pending is not None:
                        norm_head(pending)
                    for idx in range(32):
                        hk = hooks.get((h, idx))
                        if hk is not None:
                            hk()
                        u = order[idx]
                        t, half = u // 2, u % 2
                        vs = v_sb[:, (t * HPC + h) * 65:(t * HPC + h) * 65 + 65]
                        nc.tensor.matmul(puts[half][:], vs, es[u][:],
                                         start=(t == 0), stop=(t == ST - 1),
                                         skip_group_check=True)
                        es[u] = None
                        if idx + 3 < 32:
                            sc_exp(order[idx + 3])
                        if split and idx == 15:
                            norm_head(pend_h, halves=(0,))
                    pending = pend_h
                return pending

            hooks0 = {
                (0, 12): lambda: proj_block(wk_sb, bk_sb, xk, kt4, 2, khead=True),
                (0, 20): lambda: proj_block(wk_sb, bk_sb, xk, kt4, 3, khead=True),
                (2, 8): lambda: proj_block(wq_sb, bq_sb, xq, qt_sb, 2),
                (2, 24): lambda: proj_block(wq_sb, bq_sb, xq, qt_sb, 3),
            }
            pend0 = attention(0, hooks0)
            hooks1 = {(0, 16): lambda: norm_head(pend0)}
            _sts = [(1, 8), (1, 24), (2, 8), (2, 24), (3, 4), (3, 8), (3, 12), (3, 16),
                    (3, 26), (3, 30)]
            for st, (h, u) in enumerate(_sts):
                hooks1[(h, u)] = (lambda st_=st: outproj_block(st_))
            pend1 = attention(1, hooks1, split_last=True)
            norm_head(pend1, halves=(1,))
            outproj_block(10, tail=True)
            outproj_block(11, tail=True)
            for st in range(12, 16):
                outproj_block(st, tail=True)
    nc.compile()
    return nc


def _v_scatter_ap(v_sb, t):
    """AP writing a [128, 256] chan-major block into the 65-strided V layout."""
    ap = v_sb[:, t * HPC * 65: t * HPC * 65 + HPC * 65]
    return ap.rearrange("p (h j) -> p h j", h=HPC)[:, :, 0:64]


def _pack_w(w):
    """(R, N) row-chunked -> (128, (R//128)*N) SBUF layout, fp16 contiguous."""
    R, N = w.shape
    return np.ascontiguousarray(
        w.reshape(R // 128, 128, N).transpose(1, 0, 2).reshape(128, -1)).astype(np.float16)


def _prepare(q, k, v, Wq_w, Wq_b, Wk_w, Wk_b, Wv_w, Wv_b, Wo_w, Wo_b):
    f16 = np.float16
    pos = np.arange(1, S + 1, dtype=np.float32)
    theta = (BASE ** (-2.0 * np.arange(D // 2, dtype=np.float32) / D)).astype(np.float32)
    ang = theta[:, None] * pos[None, :]
    cosc = np.cos(ang).astype(f16)
    sinc = np.sin(ang).astype(f16)
    identity = np.eye(128, dtype=f16)

    per_batch = []
    for b in range(B):
        per_batch.append((
            np.ascontiguousarray(q[b].T[_PERM]).astype(f16),
            np.ascontiguousarray(k[b].T[_PERM]).astype(f16),
            np.ascontiguousarray(v[b].T).astype(f16),
        ))
    in_maps = []
    for c in range(N_CORES):
        b, g = divmod(c, GROUPS)
        rows = slice(CH * g, CH * (g + 1))
        qTb, kTb, vTb = per_batch[b]
        in_maps.append({
            "qT": qTb, "kT": kTb, "vT": vTb, "cosc": cosc, "sinc": sinc,
            "ident": identity,
            "wq": _pack_w(Wq_w[rows, :].T[_PERM]),
            "wk": _pack_w(Wk_w[rows, :].T[_PERM]),
            "wv": _pack_w(Wv_w[rows, :].T),
            "wo": _pack_w(Wo_w[:, rows].T),
            "bq": Wq_b[rows].astype(np.float32).reshape(2, 128, 1),
            "bk": Wk_b[rows].astype(np.float32).reshape(2, 128, 1),
        })
    return in_maps


def kernel(q, k, v, Wq_w, Wq_b, Wk_w, Wk_b, Wv_w, Wv_b, Wo_w, Wo_b):
    global _PROG
    args = [np.asarray(x, dtype=np.float32) for x in
            (q, k, v, Wq_w, Wq_b, Wk_w, Wk_b, Wv_w, Wv_b, Wo_w, Wo_b)]
    if _PROG is None:
        _PROG = _build()
    in_maps = _prepare(*args)
    res = run_bass_kernel_spmd(_PROG, in_maps, core_ids=list(range(N_CORES)))
    kernel.last_results = res
    Wv_b32, Wo_w32, Wo_b32 = args[8], args[9], args[10]
    const = Wv_b32 @ Wo_w32.T + Wo_b32
    out = np.empty((B, S, D), dtype=np.float32)
    for b in range(B):
        acc = res.results[GROUPS * b]["out"].astype(np.float32)
        for g in range(1, GROUPS):
            acc += res.results[GROUPS * b + g]["out"]
        out[b] = acc + const
    return out


# revision 29
# speedup vs baseline: 1.6874x; 1.6874x over previous
"""Multi-head attention (RoPE-full-dmodel variant) on 8 TRN2 NeuronCores.

Sharding: core c = (batch c//4, head-group c%4 of 4 heads).
 - W_q/W_k/W_v split column-wise by head (each core projects its 256 channels)
 - W_o split row-wise; per-core partial outputs summed on host (all-reduce at gather)

Perf-tuned fp16 design (394us baseline -> ~268us measured, NTFF-profiled):
 - Scores use 128-partition contraction: per-head K tiles zero-padded to 128
   partitions (K=64 matmuls run at ~half the row rate of K=128 on TRN2; the
   packed-Q rhs upper/lower half is nullified by the zeros).  kt tiles reuse
   the vch[4..7] SBUF slots (dead after the V projection) via pool-tag rotation.
 - DMA priority order (k+tables first for the DVE RoPE critical path, V path
   interleaved), half-S granularity; weights host-prepacked to SBUF layout.
 - RoPE quarter/half-S granular, in-place (roped halves overwrite inputs),
   pipelined with K/Q projections; kproj/qproj for s-blocks 2-3 are hooked
   into the attention MM stream.
 - Attention inner loop 3-ahead score pipelining ([128,512] single-bank score
   tiles; PSUM: 3 sc + 3 ut + 2 gen = 8 banks) so PV never waits on exp.
 - Softmax denominators via ones-column; deferred normalization (next head's
   prologue) keeps the norm chain off the PE critical path; ut split per
   query-block avoids coarse-range false deps on out-proj.
 - V bias is folded on host (attn rows sum to 1 => U = attn@V + bv exactly).
 - Out-proj hooked into att(qbp1); the last head runs half-major so the final
   norms overlap compute; staging copies ride the ACT engine.
"""
import os
import sys
from contextlib import nullcontext

for _p in ("/opt/trn_rl_repo", "/root/.axon_site/_ro/trn_rl_repo"):
    if os.path.isdir(_p) and _p not in sys.path:
        sys.path.insert(0, _p)

import numpy as np

import concourse.bacc as bacc
import concourse.tile as tile
import concourse.mybir as mybir
from concourse.bass_utils import run_bass_kernel_spmd

B, S, D = 2, 2048, 1024
H_TOT, DK = 16, 64
N_CORES, GROUPS = 8, 4
CH = 256            # channels (heads*dk) per core
KC = D // 128       # 8 d-model chunks
QB = S // 512       # 4 query s-blocks
ST = S // 128       # 16 seq tiles
HPC = 4             # heads per core
BASE = 10000.0

MM = mybir.dt.float16
F32 = mybir.dt.float32
AF = mybir.ActivationFunctionType

# even/odd permutation of the d_model axis: row r <- old d = 2r (r<512), 2(r-512)+1
_PERM = np.concatenate([np.arange(0, D, 2), np.arange(1, D, 2)])

_PROG = None


def _build(loop_n=1):
    nc = bacc.Bacc("TRN2", target_bir_lowering=False, debug=False)
    qT = nc.dram_tensor("qT", (D, S), MM, kind="ExternalInput").ap()
    kT = nc.dram_tensor("kT", (D, S), MM, kind="ExternalInput").ap()
    vT = nc.dram_tensor("vT", (D, S), MM, kind="ExternalInput").ap()
    cosc = nc.dram_tensor("cosc", (D // 2, S), MM, kind="ExternalInput").ap()
    sinc = nc.dram_tensor("sinc", (D // 2, S), MM, kind="ExternalInput").ap()
    wq = nc.dram_tensor("wq", (128, KC * CH), MM, kind="ExternalInput").ap()
    wk = nc.dram_tensor("wk", (128, KC * CH), MM, kind="ExternalInput").ap()
    wv = nc.dram_tensor("wv", (128, KC * CH), MM, kind="ExternalInput").ap()
    wo = nc.dram_tensor("wo", (128, 2 * D), MM, kind="ExternalInput").ap()
    bq = nc.dram_tensor("bq", (2, 128, 1), F32, kind="ExternalInput").ap()
    bk = nc.dram_tensor("bk", (2, 128, 1), F32, kind="ExternalInput").ap()
    ident = nc.dram_tensor("ident", (128, 128), MM, kind="ExternalInput").ap()
    out = nc.dram_tensor("out", (S, D), MM, kind="ExternalOutput").ap()

    with tile.TileContext(nc) as tc:
      with (
          tc.tile_pool(name="consts", bufs=1) as consts,
          tc.tile_pool(name="qkv", bufs=1) as qkv,
          tc.tile_pool(name="xin", bufs=1) as xin,
          tc.tile_pool(name="vc", bufs=1) as vc,
          tc.tile_pool(name="tmp", bufs=4) as tmpp,
          tc.tile_pool(name="expp", bufs=4) as expp,
          tc.tile_pool(name="misc", bufs=2) as misc,
          tc.tile_pool(name="outst", bufs=2) as outst,
          tc.tile_pool(name="ps_gen", bufs=2, space="PSUM") as ps_gen,
          tc.tile_pool(name="ps_sc", bufs=3, space="PSUM") as ps_sc,
          tc.tile_pool(name="ps_ut", bufs=3, space="PSUM") as ps_ut,
      ):
        with (tc.For_i(0, loop_n, 1, hint_engines=tuple(mybir.ALL_ENGINES))
              if loop_n > 1 else nullcontext()):
            # ---------------- static tiles ----------------
            wv_sb = consts.tile([128, KC * CH], MM, tag="wv")
            wk_sb = consts.tile([128, KC * CH], MM, tag="wk")
            wq_sb = consts.tile([128, KC * CH], MM, tag="wq")
            wo_sb = consts.tile([128, 2 * D], MM, tag="wo")
            ident_sb = consts.tile([128, 128], MM, tag="ident")
            bq_sb, bk_sb = [], []
            for c in range(2):
                t_ = consts.tile([128, 1], F32, tag=f"bq{c}", name=f"bq{c}")
                bq_sb.append(t_)
                t_ = consts.tile([128, 1], F32, tag=f"bk{c}", name=f"bk{c}")
                bk_sb.append(t_)

            v_sb = qkv.tile([128, ST * HPC * 65], MM, tag="v")
            ones_cols = v_sb[:].rearrange("p (b c) -> p b c", c=65)[:, :, 64]
            nc.vector.memset(ones_cols, 1.0)

            qt_sb = [qkv.tile([128, S], MM, tag=f"qt{c}", name=f"qt{c}") for c in range(2)]
            utq = [[qkv.tile([128, 1024], MM, tag=f"ut{q}{c}", name=f"ut{q}{c}")
                    for c in range(2)] for q in range(2)]
            cos_sb = [qkv.tile([128, S], MM, tag=f"cos{t}", name=f"cos{t}") for t in range(4)]
            sin_sb = [qkv.tile([128, S], MM, tag=f"sin{t}", name=f"sin{t}") for t in range(4)]
            xk = [xin.tile([128, S], MM, tag=f"xk{a}", name=f"xk{a}") for a in range(KC)]
            xq = [xin.tile([128, S], MM, tag=f"xq{a}", name=f"xq{a}") for a in range(KC)]
            vch = [vc.tile([128, S], MM, tag=f"vch{d}", name=f"vch{d}") for d in range(KC)]
            vt_cm = [vc.tile([128, S], MM, tag=f"vtc{c}", name=f"vtc{c}") for c in range(2)]

            # ---------------- DMA emission (priority order) ----------------
            HLV = ((0, 1024), (1024, 2048))

            def dma_half(dst_tile, src_rows, lo, hi):
                nc.sync.dma_start(dst_tile[:, lo:hi], src_rows[:, lo:hi])

            # weights prepacked on host into the SBUF layout (contiguous DMA)
            nc.sync.dma_start(wv_sb[:], wv)
            nc.sync.dma_start(ident_sb[:], ident)
            lo, hi = HLV[0]
            # K-path pair (0,2) + tables, then V half0, then rest of K half0
            for a in (0, 2):
                dma_half(xk[a], kT[128 * a:128 * (a + 1), :], lo, hi)
            for t in (0, 2):
                dma_half(cos_sb[t], cosc[128 * t:128 * (t + 1), :], lo, hi)
                dma_half(sin_sb[t], sinc[128 * t:128 * (t + 1), :], lo, hi)
            for d in range(KC):
                dma_half(vch[d], vT[128 * d:128 * (d + 1), :], lo, hi)
            for a in (1, 3):
                dma_half(xk[a], kT[128 * a:128 * (a + 1), :], lo, hi)
            for t in (1, 3):
                dma_half(cos_sb[t], cosc[128 * t:128 * (t + 1), :], lo, hi)
                dma_half(sin_sb[t], sinc[128 * t:128 * (t + 1), :], lo, hi)
            for a in (4, 6, 5, 7):
                dma_half(xk[a], kT[128 * a:128 * (a + 1), :], lo, hi)
            lo1, hi1 = HLV[1]
            for d in range(KC):
                dma_half(vch[d], vT[128 * d:128 * (d + 1), :], lo1, hi1)
            for a in (0, 2, 1, 3, 4, 6, 5, 7):
                dma_half(xq[a], qT[128 * a:128 * (a + 1), :], lo, hi)
            nc.sync.dma_start(wk_sb[:], wk)
            nc.sync.dma_start(wq_sb[:], wq)
            for c in range(2):
                nc.sync.dma_start(bq_sb[c][:], bq[c])
                nc.sync.dma_start(bk_sb[c][:], bk[c])
            for a in (0, 2, 1, 3, 4, 6, 5, 7):
                dma_half(xk[a], kT[128 * a:128 * (a + 1), :], lo1, hi1)
            for t in (0, 2, 1, 3):
                dma_half(cos_sb[t], cosc[128 * t:128 * (t + 1), :], lo1, hi1)
                dma_half(sin_sb[t], sinc[128 * t:128 * (t + 1), :], lo1, hi1)
            for a in (0, 2, 1, 3, 4, 6, 5, 7):
                dma_half(xq[a], qT[128 * a:128 * (a + 1), :], lo1, hi1)
            nc.sync.dma_start(wo_sb[:], wo)

            # ---------------- helpers ----------------
            def rope_cols(x, lo, hi):
                """RoPE in place on x[0..7] over columns [lo,hi) (DVE)."""
                w = hi - lo
                for a in (0, 1, 4, 5):
                    b_ = a + 2
                    ca, cb = a % 4, b_ % 4
                    xa, xb = x[a], x[b_]
                    t1 = tmpp.tile([128, w], MM, tag="t", name="t1")
                    nc.vector.tensor_mul(t1[:], xa[:, lo:hi], cos_sb[ca][:, lo:hi])
                    t2 = tmpp.tile([128, w], MM, tag="t", name="t2")
                    nc.vector.tensor_mul(t2[:], xb[:, lo:hi], sin_sb[ca][:, lo:hi])
                    t3 = tmpp.tile([128, w], MM, tag="t", name="t3")
                    nc.vector.tensor_mul(t3[:], xb[:, lo:hi], cos_sb[cb][:, lo:hi])
                    t4 = tmpp.tile([128, w], MM, tag="t", name="t4")
                    nc.vector.tensor_mul(t4[:], xa[:, lo:hi], sin_sb[cb][:, lo:hi])
                    nc.vector.tensor_sub(xa[:, lo:hi], t1[:], t2[:])
                    nc.vector.tensor_add(xb[:, lo:hi], t3[:], t4[:])

            def proj_block(w_sb, b_sb, src, dst, s, khead=False):
                """One 512-col projection s-block: psum over 8 d-chunks, ACT out.

                khead: dst is the list of 4 zero-padded per-head kt tiles; write
                the two 64-channel head halves separately (same partitions)."""
                for c in range(2):
                    ps = ps_gen.tile([128, 512], F32, tag="proj", name="psum")
                    for d in range(KC):
                        lhsT = w_sb[:, CH * d + 128 * c: CH * d + 128 * (c + 1)]
                        nc.tensor.matmul(ps[:], lhsT, src[d][:, 512 * s:512 * (s + 1)],
                                         start=(d == 0), stop=(d == KC - 1))
                    if khead:
                        sl = slice(512 * s, 512 * (s + 1))
                        nc.scalar.activation(dst[2 * c][0:64, sl], ps[0:64, :],
                                             AF.Identity, bias=b_sb[c][0:64])
                        nc.scalar.activation(dst[2 * c + 1][64:128, sl], ps[64:128, :],
                                             AF.Identity, bias=b_sb[c][64:128])
                    elif b_sb is None:
                        nc.scalar.copy(dst[c][:, 512 * s:512 * (s + 1)], ps[:])
                    else:
                        nc.scalar.activation(dst[c][:, 512 * s:512 * (s + 1)], ps[:],
                                             AF.Identity, bias=b_sb[c][:])

            def transpose_tile(t):
                pv = ps_gen.tile([128, CH], MM, tag="proj", name="pv")
                for c in range(2):
                    nc.tensor.transpose(pv[:, 128 * c:128 * (c + 1)],
                                        vt_cm[c][:, 128 * t:128 * (t + 1)],
                                        ident_sb[:])
                dst = _v_scatter_ap(v_sb, t)
                nc.scalar.copy(dst, pv[:])

            def outproj_block(st, tail=False):
                pos = [ps_gen.tile([128, 512], F32, tag="proj", name="po_")
                       for _ in range(2)]
                qbp_, loc = st // 8, 128 * (st % 8)
                for cc in range(2):
                    lhsT = utq[qbp_][cc][:, loc:loc + 128]
                    for nb in range(2):
                        nc.tensor.matmul(
                            pos[nb][:], lhsT,
                            wo_sb[:, D * cc + 512 * nb: D * cc + 512 * (nb + 1)],
                            start=(cc == 0), stop=(cc == 1))
                for nb in range(2):
                    stg = outst.tile([128, 512], MM, tag="stg", name="stg")
                    nc.scalar.copy(stg[:], pos[nb][:])
                    if tail:
                        for hh in range(2):
                            nc.sync.dma_start(
                                out[128 * st:128 * (st + 1),
                                    512 * nb + 256 * hh:512 * nb + 256 * (hh + 1)],
                                stg[:, 256 * hh:256 * (hh + 1)])
                    else:
                        nc.sync.dma_start(
                            out[128 * st:128 * (st + 1), 512 * nb:512 * (nb + 1)],
                            stg[:])

            # ---------------- V path (PE-first work) ----------------
            proj_block(wv_sb, None, vch, vt_cm, 0)
            proj_block(wv_sb, None, vch, vt_cm, 1)
            for t in range(8):
                transpose_tile(t)
            proj_block(wv_sb, None, vch, vt_cm, 2)
            proj_block(wv_sb, None, vch, vt_cm, 3)
            for t in range(8, 16):
                transpose_tile(t)

            # ---------------- RoPE + K/Q proj, S-half pipelined ----------------
            # Per-head K tiles, contraction zero-padded to 128 partitions:
            # K=64 matmuls run at ~half the row rate of K=128, so scores use
            # 128-partition lhsT with the unused head-half zeroed.  The rhs
            # (packed Q) upper/lower half carries the other head's data,
            # nullified by the zeros.  Tiles reuse the vch[4..7] SBUF (dead
            # after the V projection) via pool-tag rotation.
            kt4 = [vc.tile([128, S], MM, tag=f"vch{4 + h}", name=f"kt4_{h}")
                   for h in range(HPC)]
            for h in range(HPC):
                pad = slice(64, 128) if h % 2 == 0 else slice(0, 64)
                nc.gpsimd.memset(kt4[h][pad, :], 0.0)
            rope_cols(xk, 0, 512)
            rope_cols(xk, 512, 1024)
            proj_block(wk_sb, bk_sb, xk, kt4, 0, khead=True)
            proj_block(wk_sb, bk_sb, xk, kt4, 1, khead=True)
            rope_cols(xq, 0, 512)
            rope_cols(xq, 512, 1024)
            proj_block(wq_sb, bq_sb, xq, qt_sb, 0)
            proj_block(wq_sb, bq_sb, xq, qt_sb, 1)
            rope_cols(xk, 1024, 1536)
            rope_cols(xk, 1536, 2048)
            rope_cols(xq, 1024, 2048)

            # ---------------- attention ----------------
            def norm_head(pend, halves=(0, 1)):
                """Normalize a finished head's PV accumulators into utq."""
                qbp, ct, po, puts, uraws = pend
                for half in halves:
                    if uraws[half] is None:
                        u_ = misc.tile([65, 512], F32, tag="uraw", name="uraw")
                        nc.vector.tensor_copy(u_[:], puts[half][:])
                        uraws[half] = u_
                for half in halves:
                    uraw = uraws[half]
                    rec = misc.tile([1, 512], F32, tag="rec", name="rec")
                    nc.vector.reciprocal(rec[:], uraw[64:65, :])
                    bc = misc.tile([64, 512], F32, tag="bc", name="bc")
                    nc.gpsimd.partition_broadcast(bc[:], rec[:])
                    dst = utq[qbp][ct][po:po + 64, 512 * half:512 * (half + 1)]
                    nc.vector.tensor_mul(dst, uraw[0:64, :], bc[:])

            def attention(qbp, hooks=None, split_last=False):
                """hooks: dict (h, u) -> callable. Returns the last head's
                pending norm (emitted by the caller)."""
                hooks = hooks or {}
                q_lo = 1024 * qbp
                pending = None
                for h in range(HPC):
                    ct, po = h // 2, 64 * (h % 2)
                    qt_h = qt_sb[ct][:, q_lo:q_lo + 1024]
                    kt_h = kt4[h]
                    split = split_last and h == HPC - 1
                    order = ([2 * t for t in range(16)] + [2 * t + 1 for t in range(16)]
                             if split else list(range(32)))
                    puts = [ps_ut.tile([65, 512], F32, tag="ut", name=f"put{half}")
                            for half in range(2)]
                    pend_h = (qbp, ct, po, puts, [None, None])
                    es = [None] * 32

                    def sc_exp(u):
                        t, half = u // 2, u % 2
                        psc = ps_sc.tile([128, 512], F32, tag="sc", name="psc")
                        nc.tensor.matmul(psc[:],
                                         kt_h[:, 128 * t:128 * (t + 1)],
                                         qt_h[:, 512 * half:512 * (half + 1)],
                                         start=True, stop=True)
                        e = expp.tile([128, 512], MM, tag="e", name="e")
                        nc.scalar.activation(e[:], psc[:], AF.Exp, scale=0.125)
                        es[u] = e

                    sc_exp(order[0]); sc_exp(order[1]); sc_exp(order[2])
                    if pending is not None:
                        norm_head(pending)
                    for idx in range(32):
                        hk = hooks.get((h, idx))
                        if hk is not None:
                            hk()
                        u = order[idx]
                        t, half = u // 2, u % 2
                        vs = v_sb[:, (t * HPC + h) * 65:(t * HPC + h) * 65 + 65]
                        nc.tensor.matmul(puts[half][:], vs, es[u][:],
                                         start=(t == 0), stop=(t == ST - 1),
                                         skip_group_check=True)
                        es[u] = None
                        if idx + 3 < 32:
                            sc_exp(order[idx + 3])
                        if split and idx == 15:
                            norm_head(pend_h, halves=(0,))
                    pending = pend_h
                return pending

            hooks0 = {
                (0, 12): lambda: proj_block(wk_sb, bk_sb, xk, kt4, 2, khead=True),
                (0, 20): lambda: proj_block(wk_sb, bk_sb, xk, kt4, 3, khead=True),
                (2, 8): lambda: proj_block(wq_sb, bq_sb, xq, qt_sb, 2),
                (2, 24): lambda: proj_block(wq_sb, bq_sb, xq, qt_sb, 3),
            }
            pend0 = attention(0, hooks0)
            hooks1 = {(0, 16): lambda: norm_head(pend0)}
            _sts = [(1, 8), (1, 24), (2, 8), (2, 24), (3, 4), (3, 8), (3, 12), (3, 16),
                    (3, 26), (3, 30)]
            for st, (h, u) in enumerate(_sts):
                hooks1[(h, u)] = (lambda st_=st: outproj_block(st_))
            pend1 = attention(1, hooks1, split_last=True)
            norm_head(pend1, halves=(1,))
            outproj_block(10, tail=True)
            outproj_block(11, tail=True)
            for st in range(12, 16):
                outproj_block(st, tail=True)
    nc.compile()
    return nc


def _v_scatter_ap(v_sb, t):
    """AP writing a [128, 256] chan-major block into the 65-strided V layout."""
    ap = v_sb[:, t * HPC * 65: t * HPC * 65 + HPC * 65]
    return ap.rearrange("p (h j) -> p h j", h=HPC)[:, :, 0:64]


def _pack_w(w):
    """(R, N) row-chunked -> (128, (R//128)*N) SBUF layout, fp16 contiguous."""
    R, N = w.shape
    return np.ascontiguousarray(
        w.reshape(R // 128, 128, N).transpose(1, 0, 2).reshape(128, -1)).astype(np.float16)


def _prepare(q, k, v, Wq_w, Wq_b, Wk_w, Wk_b, Wv_w, Wv_b, Wo_w, Wo_b):
    f16 = np.float16
    pos = np.arange(1, S + 1, dtype=np.float32)
    theta = (BASE ** (-2.0 * np.arange(D // 2, dtype=np.float32) / D)).astype(np.float32)
    ang = theta[:, None] * pos[None, :]
    cosc = np.cos(ang).astype(f16)
    sinc = np.sin(ang).astype(f16)
    identity = np.eye(128, dtype=f16)

    per_batch = []
    for b in range(B):
        per_batch.append((
            np.ascontiguousarray(q[b].T[_PERM]).astype(f16),
            np.ascontiguousarray(k[b].T[_PERM]).astype(f16),
            np.ascontiguousarray(v[b].T).astype(f16),
        ))
    in_maps = []
    for c in range(N_CORES):
        b, g = divmod(c, GROUPS)
        rows = slice(CH * g, CH * (g + 1))
        qTb, kTb, vTb = per_batch[b]
        in_maps.append({
            "qT": qTb, "kT": kTb, "vT": vTb, "cosc": cosc, "sinc": sinc,
            "ident": identity,
            "wq": _pack_w(Wq_w[rows, :].T[_PERM]),
            "wk": _pack_w(Wk_w[rows, :].T[_PERM]),
            "wv": _pack_w(Wv_w[rows, :].T),
            "wo": _pack_w(Wo_w[:, rows].T),
            "bq": Wq_b[rows].astype(np.float32).reshape(2, 128, 1),
            "bk": Wk_b[rows].astype(np.float32).reshape(2, 128, 1),
        })
    return in_maps


def kernel(q, k, v, Wq_w, Wq_b, Wk_w, Wk_b, Wv_w, Wv_b, Wo_w, Wo_b):
    global _PROG
    args = [np.asarray(x, dtype=np.float32) for x in
            (q, k, v, Wq_w, Wq_b, Wk_w, Wk_b, Wv_w, Wv_b, Wo_w, Wo_b)]
    if _PROG is None:
        _PROG = _build()
    in_maps = _prepare(*args)
    res = run_bass_kernel_spmd(_PROG, in_maps, core_ids=list(range(N_CORES)))
    kernel.last_results = res
    Wv_b32, Wo_w32, Wo_b32 = args[8], args[9], args[10]
    const = Wv_b32 @ Wo_w32.T + Wo_b32
    out = np.empty((B, S, D), dtype=np.float32)
    for b in range(B):
        acc = res.results[GROUPS * b]["out"].astype(np.float32)
        for g in range(1, GROUPS):
            acc += res.results[GROUPS * b + g]["out"]
        out[b] = acc + const
    return out
